# revision 14
# baseline (speedup 1.0000x reference)
"""Multi-head attention (B=4, S=2048, E=1024, H=16) on 8 NeuronCores.

Sharding: (batch, head-half). Core c handles batch c//2 and heads
(c%2)*8..(c%2)*8+8 over the FULL sequence: Wq/Wk/Wv column-sliced,
Wo row-sliced. Each core emits a partial output projection; the pair
partials (and the rank-1 bias row bv@Wo + bo) are summed on the host,
so no device collectives and no duplicated projection work.

Per-core program is a single fused pipeline: projection (PV/PK/PQ),
attention, and output-projection matmuls are interleaved in one PE
stream so the PE never idles while ACT computes exp. ACT is reserved
exclusively for Exp (everything else runs on DVE/Pool), since exp of
33.5M scores (~257us at 1 elem/lane/cycle) is the co-bottleneck.

  PV  V = (vinT^T Wv) in [k, e] layout, parity-packed per head with a
      ones column for the softmax denominator (even h: [V|1|0], odd h:
      [1|0|V]) so ctx lands on the head's own partitions.
  PK  K^T = Wk^T kinT + bk   ([e, k] layout, resident)
  PQ  Q^T = Wq^T qinT + bq   ([e, q] layout, zero-padded per head)
  PA  per (qc, h), streaming k-tile pairs:
        S^T  = K_h^T-tiles @ Q_h^T          (PSUM, k on partitions)
        expS = Exp(S^T * 1/8)               (no max subtraction: ~N(0,1))
        ctx_unT/den = V_aug^T @ expS        (PSUM accum over k tiles)
        ctx^T = ctx_unT * bcast(1/den)      (recip_approx_fast on DVE)
  PO  out_partial = ctx^T-tiles^T @ Wo_rows (bias added on host)
"""

import os
import sys

for _p in ("/opt/trn_rl_repo", os.path.expanduser("~/.axon_site/_ro/trn_rl_repo")):
    if os.path.isdir(_p) and _p not in sys.path:
        sys.path.append(_p)

import numpy as np

import concourse.bass as bass
import concourse.tile as tile
from concourse import bacc, mybir
from concourse.bass_utils import run_bass_kernel_spmd

E = 1024  # embed dim (contraction for QKV projections)
HL = 8  # heads per core
D = 64
S = 2048  # sequence length (full q and k per core)
EO = 512  # local e-out = HL * D
P = 128
KE = E // P  # 8 e-in tiles
ETO = EO // P  # 4 local e-out tiles
NKT = S // P  # 16 k tiles
NQC = S // 512  # 4 q chunks
B = 4
F32 = mybir.dt.float32
BF16 = mybir.dt.bfloat16
N_CORES = 8

_CACHE = {}
_LAST_RESULTS = None


def _build_program():
    nc = bacc.Bacc("TRN2", target_bir_lowering=False, debug=False, num_devices=N_CORES)

    qinT_d = nc.dram_tensor("qinT", [E, S], BF16, kind="ExternalInput").ap()
    kinT_d = nc.dram_tensor("kinT", [E, S], BF16, kind="ExternalInput").ap()
    vinT_d = nc.dram_tensor("vinT", [E, S], BF16, kind="ExternalInput").ap()
    Wq = nc.dram_tensor("Wq", [E, EO], BF16, kind="ExternalInput").ap()
    Wk = nc.dram_tensor("Wk", [E, EO], BF16, kind="ExternalInput").ap()
    Wv = nc.dram_tensor("Wv", [E, EO], BF16, kind="ExternalInput").ap()
    Wo = nc.dram_tensor("Wo", [EO, E], BF16, kind="ExternalInput").ap()
    bq = nc.dram_tensor("bq", [EO], F32, kind="ExternalInput").ap()
    bk = nc.dram_tensor("bk", [EO], F32, kind="ExternalInput").ap()
    out = nc.dram_tensor("out", [S, E], F32, kind="ExternalOutput").ap()

    with tile.TileContext(nc) as tc:
        with (
            tc.tile_pool(name="const", bufs=1) as const,
            tc.tile_pool(name="persist", bufs=1) as persist,
            tc.tile_pool(name="w", bufs=1) as wpool,
            tc.tile_pool(name="vin", bufs=4) as vin_pool,
            tc.tile_pool(name="kin", bufs=3) as kin_pool,
            tc.tile_pool(name="qin", bufs=2) as qin_pool,
            tc.tile_pool(name="exp", bufs=4) as exp_pool,
            tc.tile_pool(name="nrm", bufs=2) as nrm_pool,
            tc.tile_pool(name="outs", bufs=2) as out_pool,
            tc.tile_pool(name="sps", bufs=2, space="PSUM") as s_psum,
            tc.tile_pool(name="cps", bufs=2, space="PSUM") as c_psum,
            tc.tile_pool(name="gps", bufs=2, space="PSUM") as g_psum,
        ):
            # ---- constants / weights -----------------------------------
            bq_sb = const.tile([P, ETO], F32)
            nc.sync.dma_start(out=bq_sb[:], in_=bq.rearrange("(t p) -> p t", p=P))
            bk_sb = const.tile([P, ETO], F32)
            nc.sync.dma_start(out=bk_sb[:], in_=bk.rearrange("(t p) -> p t", p=P))

            # weight tiles; DMAs are scheduled inside the plan ('dw' items)
            wv_sb = wpool.tile([P, KE, EO], BF16, tag="wv")
            wk_sb = wpool.tile([P, KE, EO], BF16, tag="wk")
            wq_sb = wpool.tile([P, KE, EO], BF16, tag="wq")
            wo_sb = wpool.tile([P, ETO, E], BF16, tag="wo")
            w_dma = {
                "wv": (wv_sb, Wv.rearrange("(ke p) e -> p ke e", p=P)),
                "wk": (wk_sb, Wk.rearrange("(ke p) e -> p ke e", p=P)),
                "wq": (wq_sb, Wq.rearrange("(ke p) e -> p ke e", p=P)),
                "wo": (wo_sb, Wo.rearrange("(ke p) e -> p ke e", p=P)),
            }

            ones_scr = const.tile([P, NKT * (HL // 2)], F32)
            nc.vector.memset(ones_scr[:], 1.0)

            # V_aug parity-packed: even h: [V(0:64)|ones@64|0]; odd h:
            # [ones@0|0|V(64:128)]
            v_sb = persist.tile([P, NKT, HL, P], BF16)
            v_g = v_sb.rearrange("p kt (hp two) c -> p kt hp two c", two=2)
            nc.vector.tensor_copy(
                v_g[:, :, :, 0, D],
                ones_scr[:].rearrange("p (a b) -> p a b", a=NKT),
            )
            nc.vector.tensor_copy(
                v_g[:, :, :, 1, 0],
                ones_scr[:].rearrange("p (a b) -> p a b", a=NKT),
            )
            nc.vector.memset(v_g[:, :, :, 0, D + 1 : P], 0.0)
            nc.gpsimd.memset(v_g[:, :, :, 1, 1:D], 0.0)

            # resident K^T (heads packed 2/tile) and padded per-head Q^T
            kT = persist.tile([P, ETO, S], BF16)  # 16KB/part
            qT = persist.tile([P, HL, S], BF16)  # 32KB/part
            qT_g = qT.rearrange("p (hp two) k -> p hp two k", two=2)
            nc.vector.memset(qT_g[D:P, 0:2, 0, :], 0.0)
            nc.gpsimd.memset(qT_g[D:P, 2:4, 0, :], 0.0)
            nc.vector.memset(qT_g[0:D, 0:2, 1, :], 0.0)
            nc.gpsimd.memset(qT_g[0:D, 2:4, 1, :], 0.0)

            # ctx accumulates in SBUF (bf16), feeds PO
            ctx_sb = persist.tile([P, ETO, S], BF16)  # 16KB/part

            # ---- production closures (emitted interleaved with PA) -----
            # PSUM is reachable only from PE/ACT/DVE, so all PSUM drains go
            # to DVE, except half the PK/PQ drains which use ACT (they all
            # land in qc0 where exp leaves the scalar engine slack).

            vinT_r = vinT_d.rearrange("(ke p) s -> p ke s", p=P)
            kinT_r = kinT_d.rearrange("(ke p) s -> p ke s", p=P)
            qinT_r = qinT_d.rearrange("(ke p) s -> p ke s", p=P)
            vin_bufs = {}

            def dma_vin(pair):  # stage vinT k-tiles 2*pair, 2*pair+1
                t = vin_pool.tile([P, KE, 2 * P], BF16, tag="vin")
                nc.sync.dma_start(
                    out=t[:], in_=vinT_r[:, :, pair * 2 * P : (pair + 1) * 2 * P]
                )
                vin_bufs[pair] = t

            def pv(kt):  # project V k-tile kt (all local heads)
                t = vin_bufs[kt // 2]
                sub = (kt % 2) * P
                ps = g_psum.tile([P, EO], F32, tag="g")
                for ke in range(KE):
                    nc.tensor.matmul(
                        ps[:],
                        lhsT=t[:, ke, sub : sub + P],
                        rhs=wv_sb[:, ke, :],
                        start=(ke == 0),
                        stop=(ke == KE - 1),
                    )
                ps_g = ps[:].rearrange("p (h2 two d) -> p h2 two d", h2=HL // 2, two=2)
                nc.vector.tensor_copy(v_g[:, kt, :, 0, 0:D], ps_g[:, :, 0, :])
                nc.vector.tensor_copy(v_g[:, kt, :, 1, D:P], ps_g[:, :, 1, :])

            kin_bufs = {}

            def dma_kin(c):  # stage kinT cols c*512..(c+1)*512
                t = kin_pool.tile([P, KE, 512], BF16, tag="kin")
                nc.sync.dma_start(out=t[:], in_=kinT_r[:, :, c * 512 : (c + 1) * 512])
                kin_bufs[c] = t

            def pk(et, c):  # K^T e-tile et for k chunk c
                t = kin_bufs[c]
                ps = g_psum.tile([P, 512], F32, tag="g")
                for ke in range(KE):
                    nc.tensor.matmul(
                        ps[:],
                        lhsT=wk_sb[:, ke, et * P : (et + 1) * P],
                        rhs=t[:, ke, :],
                        start=(ke == 0),
                        stop=(ke == KE - 1),
                    )
                if (et + c) % 2 == 0:
                    nc.vector.tensor_scalar_add(
                        kT[:, et, c * 512 : (c + 1) * 512],
                        ps[:],
                        bk_sb[:, et : et + 1],
                    )
                else:
                    nc.scalar.activation(
                        kT[:, et, c * 512 : (c + 1) * 512],
                        ps[:],
                        mybir.ActivationFunctionType.Identity,
                        bias=bk_sb[:, et : et + 1],
                    )

            qin_bufs = {}

            def dma_qin(c):
                t = qin_pool.tile([P, KE, 512], BF16, tag="qin")
                nc.sync.dma_start(out=t[:], in_=qinT_r[:, :, c * 512 : (c + 1) * 512])
                qin_bufs[c] = t

            def pq(et, c):  # Q^T for heads (2et, 2et+1), q chunk c
                t = qin_bufs[c]
                ps = g_psum.tile([P, 512], F32, tag="g")
                for ke in range(KE):
                    nc.tensor.matmul(
                        ps[:],
                        lhsT=wq_sb[:, ke, et * P : (et + 1) * P],
                        rhs=t[:, ke, :],
                        start=(ke == 0),
                        stop=(ke == KE - 1),
                    )
                nc.vector.tensor_scalar_add(
                    qT[0:D, 2 * et, c * 512 : (c + 1) * 512],
                    ps[0:D, :],
                    bq_sb[0:D, et : et + 1],
                )
                nc.scalar.activation(
                    qT[D:P, 2 * et + 1, c * 512 : (c + 1) * 512],
                    ps[D:P, :],
                    mybir.ActivationFunctionType.Identity,
                    bias=bq_sb[D:P, et : et + 1],
                )

            def po(qt, ch):  # partial out rows qt*128, cols ch*512
                ps = g_psum.tile([P, 512], F32, tag="g")
                for ke in range(ETO):
                    nc.tensor.matmul(
                        ps[:],
                        lhsT=ctx_sb[:, ke, qt * P : (qt + 1) * P],
                        rhs=wo_sb[:, ke, ch * 512 : (ch + 1) * 512],
                        start=(ke == 0),
                        stop=(ke == ETO - 1),
                    )
                ot = out_pool.tile([P, 512], F32, tag="out_t")
                nc.vector.tensor_copy(ot[:], ps[:])
                nc.sync.dma_start(
                    out=out[qt * P : (qt + 1) * P, ch * 512 : (ch + 1) * 512],
                    in_=ot[:],
                )

            # ---- interleave plan ---------------------------------------
            # Production items run on the PE between PA steps; each PA step
            # is gated (by tile deps) on the productions emitted before it.
            NK2 = NKT // 2

            plan = []  # 'd*' = dma, tuple = production, ('pa',...) = step

            def pa_steps(qc, h, prods):
                # 8 PA steps for group (qc, h) with prods[i] emitted before
                # step i
                for j in range(NK2):
                    for it in prods[j] if j < len(prods) else []:
                        plan.append(it)
                    plan.append(("pa", h, qc, j))

            # prefix: minimal DMA critical path to the first PA step, then
            # the rest of the staging queue (sync engine issues in order)
            plan += [("dw", "wv"), ("dv", 0), ("dwp", "wk", 0), ("dk", 0),
                     ("dwp", "wq", 0), ("dq", 0), ("dv", 1), ("dv", 2), ("dv", 3),
                     ("dk", 1),
                     ("dwp", "wk", 1), ("dwp", "wk", 2), ("dwp", "wk", 3),
                     ("dwp", "wq", 1), ("dwp", "wq", 2), ("dwp", "wq", 3)]
            plan += [("pv", 0), ("pv", 1), ("pk", 0, 0), ("pq", 0, 0)]
            # qc0 h0: step j consumes PV kt 2j,2j+1 and kT[0] chunk j//2;
            # PK produced c-major-ish so kin staging buffers rotate in order
            pa_steps(0, 0, [
                [("dv", 4), ("pv", 2), ("pv", 3), ("pk", 0, 1)],
                [("dv", 5), ("pv", 4), ("pv", 5), ("dk", 2), ("pk", 1, 0),
                 ("pk", 1, 1)],
                [("dv", 6), ("pv", 6), ("pv", 7), ("pk", 2, 0), ("pk", 2, 1)],
                [("dv", 7), ("pv", 8), ("pv", 9), ("pk", 3, 0), ("pk", 0, 2)],
                [("pv", 10), ("pv", 11), ("dk", 3), ("pk", 3, 1), ("pk", 1, 2)],
                [("pv", 12), ("pv", 13), ("pk", 2, 2), ("pk", 0, 3)],
                [("pv", 14), ("pv", 15), ("pk", 3, 2), ("pk", 1, 3)],
                [("pk", 2, 3)],
            ])
            pa_steps(0, 1, [[("pk", 3, 3)], [("pq", 1, 0)], [("pq", 2, 0)],
                            [("dw", "wo")], [], [], [], []])
            pa_steps(0, 2, [[("dq", 1)], [], [], [], [], [], [], []])
            pa_steps(0, 3, [[("pq", 3, 0)], [], [("pq", 0, 1)], [],
                            [("pq", 1, 1)], [], [("pq", 2, 1)], []])
            pa_steps(0, 4, [[("pq", 3, 1)], [], [("dq", 2)], [], [], [], [], []])
            pa_steps(0, 5, [[("pq", 0, 2)], [], [("pq", 1, 2)], [],
                            [("pq", 2, 2)], [], [("pq", 3, 2)], []])
            pa_steps(0, 6, [[("dq", 3)], [], [], [], [], [], [], []])
            pa_steps(0, 7, [[("pq", 0, 3)], [], [("pq", 1, 3)], [],
                            [("pq", 2, 3)], [], [("pq", 3, 3)], []])
            # qc1..3: interleave PO of the previous (completed) q chunk.
            # PO goes at step>=1 so the previous group's pending PV flush
            # (emitted inside step 0) precedes it in the PE stream.
            for qc in range(1, NQC):
                for h in range(HL):
                    qt0 = (qc - 1) * 4
                    prods = []
                    if h < 4:
                        prods = [[], [("po", qt0 + h, 0)], [], [],
                                 [("po", qt0 + h, 1)], [], [], []]
                    pa_steps(qc, h, prods)
            # tail: flush the last pending PV/normalize, then PO of qc3
            plan.append(("flush",))
            for qt in range(12, 16):
                for ch in range(2):
                    plan.append(("po", qt, ch))

            # ---- emit --------------------------------------------------
            pending = None  # (exp_t, h, qc, kt2, ctx_ps)
            ctx_map = {}

            def emit_pv_mm(exp_t, h, qc, kt2, ctx_ps):
                for j in range(2):
                    nc.tensor.matmul(
                        ctx_ps[:],
                        lhsT=v_sb[:, 2 * kt2 + j, h, :],
                        rhs=exp_t[:, j, :],
                        start=(kt2 == 0 and j == 0),
                        stop=(kt2 == NK2 - 1 and j == 1),
                    )  # out rows: ctx on head's parity, denom on the other
                if kt2 == NK2 - 1:
                    par = h % 2
                    dlo, dhi = (0, D) if par == 0 else (D, P)
                    den_row = D if par == 0 else 0
                    recip = nrm_pool.tile([1, 512], F32, tag="recip")
                    nc.vector.reciprocal(recip[:], ctx_ps[den_row : den_row + 1, :])
                    rb = nrm_pool.tile([P, 512], F32, tag="rb")
                    nc.gpsimd.partition_broadcast(rb[:], recip[:])
                    nc.vector.tensor_mul(
                        ctx_sb[dlo:dhi, h // 2, qc * 512 : (qc + 1) * 512],
                        ctx_ps[dlo:dhi, :],
                        rb[dlo:dhi, :],
                    )

            for it in plan:
                kind = it[0]
                if kind == "dv":
                    dma_vin(it[1])
                elif kind == "dk":
                    dma_kin(it[1])
                elif kind == "dq":
                    dma_qin(it[1])
                elif kind == "dw":
                    t, src = w_dma[it[1]]
                    nc.sync.dma_start(out=t[:], in_=src)
                elif kind == "dwp":
                    t, src = w_dma[it[1]]
                    et = it[2]
                    nc.sync.dma_start(
                        out=t[:, :, et * P : (et + 1) * P],
                        in_=src[:, :, et * P : (et + 1) * P],
                    )
                elif kind == "flush":
                    if pending is not None:
                        emit_pv_mm(*pending)
                        pending = None
                elif kind == "pv":
                    pv(it[1])
                elif kind == "pk":
                    pk(it[1], it[2])
                elif kind == "pq":
                    pq(it[1], it[2])
                elif kind == "po":
                    po(it[1], it[2])
                else:  # PA step
                    _, h, qc, kt2 = it
                    if kt2 == 0:
                        ctx_map[(h, qc)] = c_psum.tile(
                            [P, 512], F32, tag="ctx", name="ctx_ps"
                        )
                    ctx_ps = ctx_map[(h, qc)]
                    s_ps = s_psum.tile([P, 2, 512], F32, tag="s")
                    for j in range(2):
                        nc.tensor.matmul(
                            s_ps[:, j, :],
                            lhsT=kT[
                                :, h // 2, (2 * kt2 + j) * P : (2 * kt2 + j + 1) * P
                            ],
                            rhs=qT[:, h, qc * 512 : (qc + 1) * 512],
                            start=True,
                            stop=True,
                        )
                    exp_t = exp_pool.tile([P, 2, 512], BF16, tag="exp")
                    nc.scalar.activation(
                        exp_t[:],
                        s_ps[:],
                        mybir.ActivationFunctionType.Exp,
                        scale=0.125,
                    )
                    if pending is not None:
                        emit_pv_mm(*pending)
                    pending = (exp_t, h, qc, kt2, ctx_ps)
            if pending is not None:
                emit_pv_mm(*pending)

    nc.compile()
    return nc


def _get_program():
    if "nc" not in _CACHE:
        _CACHE["nc"] = _build_program()
    return _CACHE["nc"]


def kernel(query, key, value, Wq, Wk, Wv, Wo, bq, bk, bv, bo):
    global _LAST_RESULTS
    import ml_dtypes

    bf16 = ml_dtypes.bfloat16
    query = np.asarray(query, dtype=np.float32)
    key = np.asarray(key, dtype=np.float32)
    value = np.asarray(value, dtype=np.float32)
    Wq = np.asarray(Wq, np.float32)
    Wk = np.asarray(Wk, np.float32)
    Wv = np.asarray(Wv, np.float32)
    Wo = np.asarray(Wo, np.float32)

    in_maps = []
    for c in range(N_CORES):
        b, hh = c // 2, c % 2
        cs = slice(hh * EO, (hh + 1) * EO)
        in_maps.append(
            {
                "qinT": np.ascontiguousarray(query[b].T.astype(bf16)),
                "kinT": np.ascontiguousarray(key[b].T.astype(bf16)),
                "vinT": np.ascontiguousarray(value[b].T.astype(bf16)),
                "Wq": np.ascontiguousarray(Wq[:, cs].astype(bf16)),
                "Wk": np.ascontiguousarray(Wk[:, cs].astype(bf16)),
                "Wv": np.ascontiguousarray(Wv[:, cs].astype(bf16)),
                "Wo": np.ascontiguousarray(Wo[cs, :].astype(bf16)),
                "bq": np.ascontiguousarray(np.asarray(bq, np.float32)[cs]),
                "bk": np.ascontiguousarray(np.asarray(bk, np.float32)[cs]),
            }
        )
    nc = _get_program()
    res = run_bass_kernel_spmd(nc, in_maps, list(range(N_CORES)))
    _LAST_RESULTS = res
    row = (
        np.asarray(bv, np.float64) @ np.asarray(Wo, np.float64)
        + np.asarray(bo, np.float64)
    ).astype(np.float32)
    full = np.empty((B, S, E), dtype=np.float32)
    for b in range(B):
        full[b] = res.results[2 * b]["out"] + res.results[2 * b + 1]["out"] + row
    return full


# revision 17
# speedup vs baseline: 1.1043x; 1.1043x over previous
"""Multi-head attention (B=4, S=2048, E=1024, H=16) on 8 NeuronCores.

Sharding: (batch, head-half). Core c handles batch c//2 and heads
(c%2)*8..(c%2)*8+8 over the FULL sequence: Wq/Wk/Wv column-sliced,
Wo row-sliced. Each core emits a partial output projection; the pair
partials (and the rank-1 bias row bv@Wo + bo) are summed on the host,
so no device collectives and no duplicated projection work.

Per-core program is a single fused pipeline: projection (PV/PK/PQ),
attention, and output-projection matmuls are interleaved in one PE
stream so the PE never idles while ACT computes exp. ACT is reserved
exclusively for Exp (everything else runs on DVE/Pool), since exp of
33.5M scores (~257us at 1 elem/lane/cycle) is the co-bottleneck.

  PV  V = (vinT^T Wv) in [k, e] layout, parity-packed per head with a
      ones column for the softmax denominator (even h: [V|1|0], odd h:
      [1|0|V]) so ctx lands on the head's own partitions.
  PK  K^T = Wk^T kinT + bk   ([e, k] layout, resident)
  PQ  Q^T = Wq^T qinT + bq   ([e, q] layout, zero-padded per head)
  PA  per (qc, h), streaming k-tile pairs:
        S^T  = K_h^T-tiles @ Q_h^T          (PSUM, k on partitions)
        expS = Exp(S^T * 1/8)               (no max subtraction: ~N(0,1))
        ctx_unT/den = V_aug^T @ expS        (PSUM accum over k tiles)
        ctx^T = ctx_unT * bcast(1/den)      (recip_approx_fast on DVE)
  PO  out_partial = ctx^T-tiles^T @ Wo_rows (bias added on host)
"""

import os
import sys

for _p in ("/opt/trn_rl_repo", os.path.expanduser("~/.axon_site/_ro/trn_rl_repo")):
    if os.path.isdir(_p) and _p not in sys.path:
        sys.path.append(_p)

import numpy as np

import concourse.bass as bass
import concourse.tile as tile
from concourse import bacc, mybir
from concourse.bass_utils import run_bass_kernel_spmd

E = 1024  # embed dim (contraction for QKV projections)
HL = 8  # heads per core
D = 64
S = 2048  # sequence length (full q and k per core)
EO = 512  # local e-out = HL * D
P = 128
KE = E // P  # 8 e-in tiles
ETO = EO // P  # 4 local e-out tiles
NKT = S // P  # 16 k tiles
NQC = S // 512  # 4 q chunks
B = 4
F32 = mybir.dt.float32
BF16 = mybir.dt.bfloat16
N_CORES = 8

_CACHE = {}
_LAST_RESULTS = None


def _build_program():
    nc = bacc.Bacc("TRN2", target_bir_lowering=False, debug=False, num_devices=N_CORES)

    qinT_d = nc.dram_tensor("qinT", [E, S], BF16, kind="ExternalInput").ap()
    kinT_d = nc.dram_tensor("kinT", [E, S], BF16, kind="ExternalInput").ap()
    vinT_d = nc.dram_tensor("vinT", [E, S], BF16, kind="ExternalInput").ap()
    Wq = nc.dram_tensor("Wq", [E, EO], BF16, kind="ExternalInput").ap()
    Wk = nc.dram_tensor("Wk", [E, EO], BF16, kind="ExternalInput").ap()
    Wv = nc.dram_tensor("Wv", [E, EO], BF16, kind="ExternalInput").ap()
    Wo = nc.dram_tensor("Wo", [EO, E], BF16, kind="ExternalInput").ap()
    bq = nc.dram_tensor("bq", [EO], F32, kind="ExternalInput").ap()
    bk = nc.dram_tensor("bk", [EO], F32, kind="ExternalInput").ap()
    out = nc.dram_tensor("out", [S, E], F32, kind="ExternalOutput").ap()

    with tile.TileContext(nc) as tc:
        with (
            tc.tile_pool(name="const", bufs=1) as const,
            tc.tile_pool(name="persist", bufs=1) as persist,
            tc.tile_pool(name="w", bufs=1) as wpool,
            tc.tile_pool(name="vin", bufs=4) as vin_pool,
            tc.tile_pool(name="kin", bufs=3) as kin_pool,
            tc.tile_pool(name="qin", bufs=2) as qin_pool,
            tc.tile_pool(name="exp", bufs=3) as exp_pool,
            tc.tile_pool(name="nrm", bufs=2) as nrm_pool,
            tc.tile_pool(name="outs", bufs=2) as out_pool,
            tc.tile_pool(name="sps", bufs=2, space="PSUM") as s_psum,
            tc.tile_pool(name="cps", bufs=2, space="PSUM") as c_psum,
            tc.tile_pool(name="gps", bufs=2, space="PSUM") as g_psum,
        ):
            # ---- constants / weights -----------------------------------
            bq_sb = const.tile([P, ETO], F32)
            nc.sync.dma_start(out=bq_sb[:], in_=bq.rearrange("(t p) -> p t", p=P))
            bk_sb = const.tile([P, ETO], F32)
            nc.sync.dma_start(out=bk_sb[:], in_=bk.rearrange("(t p) -> p t", p=P))

            # weight tiles; DMAs are scheduled inside the plan ('dw' items)
            wv_sb = wpool.tile([P, KE, EO], BF16, tag="wv")
            wk_sb = wpool.tile([P, KE, EO], BF16, tag="wk")
            wq_sb = wpool.tile([P, KE, EO], BF16, tag="wq")
            wo_sb = wpool.tile([P, ETO, E], BF16, tag="wo")
            w_dma = {
                "wv": (wv_sb, Wv.rearrange("(ke p) e -> p ke e", p=P)),
                "wk": (wk_sb, Wk.rearrange("(ke p) e -> p ke e", p=P)),
                "wq": (wq_sb, Wq.rearrange("(ke p) e -> p ke e", p=P)),
                "wo": (wo_sb, Wo.rearrange("(ke p) e -> p ke e", p=P)),
            }

            ones_scr = const.tile([P, NKT * (HL // 2)], F32)
            nc.vector.memset(ones_scr[:], 1.0)

            # V_aug parity-packed: even h: [V(0:64)|ones@64|0]; odd h:
            # [ones@0|0|V(64:128)]
            v_sb = persist.tile([P, NKT, HL, P], BF16)
            v_g = v_sb.rearrange("p kt (hp two) c -> p kt hp two c", two=2)
            nc.vector.tensor_copy(
                v_g[:, :, :, 0, D],
                ones_scr[:].rearrange("p (a b) -> p a b", a=NKT),
            )
            nc.vector.tensor_copy(
                v_g[:, :, :, 1, 0],
                ones_scr[:].rearrange("p (a b) -> p a b", a=NKT),
            )
            nc.vector.memset(v_g[:, :, :, 0, D + 1 : P], 0.0)
            nc.gpsimd.memset(v_g[:, :, :, 1, 1:D], 0.0)

            # resident K^T (heads packed 2/tile) and padded per-head Q^T
            kT = persist.tile([P, ETO, S], BF16)  # 16KB/part
            qT = persist.tile([P, HL, S], BF16)  # 32KB/part
            qT_g = qT.rearrange("p (hp two) k -> p hp two k", two=2)
            nc.vector.memset(qT_g[D:P, 0:2, 0, :], 0.0)
            nc.gpsimd.memset(qT_g[D:P, 2:4, 0, :], 0.0)
            nc.vector.memset(qT_g[0:D, 0:2, 1, :], 0.0)
            nc.gpsimd.memset(qT_g[0:D, 2:4, 1, :], 0.0)

            # ctx accumulates in SBUF (bf16), feeds PO
            ctx_sb = persist.tile([P, ETO, S], BF16)  # 16KB/part

            # ---- production closures (emitted interleaved with PA) -----
            # PSUM is reachable only from PE/ACT/DVE, so all PSUM drains go
            # to DVE, except half the PK/PQ drains which use ACT (they all
            # land in qc0 where exp leaves the scalar engine slack).

            vinT_r = vinT_d.rearrange("(ke p) s -> p ke s", p=P)
            kinT_r = kinT_d.rearrange("(ke p) s -> p ke s", p=P)
            qinT_r = qinT_d.rearrange("(ke p) s -> p ke s", p=P)
            vin_bufs = {}

            def dma_vin(pair, eng=None):  # stage vinT k-tiles 2p, 2p+1
                t = vin_pool.tile([P, KE, 2 * P], BF16, tag="vin")
                (eng or nc.sync).dma_start(
                    out=t[:], in_=vinT_r[:, :, pair * 2 * P : (pair + 1) * 2 * P]
                )
                vin_bufs[pair] = t

            def pv(kt):  # project V k-tile kt (all local heads)
                t = vin_bufs[kt // 2]
                sub = (kt % 2) * P
                ps = g_psum.tile([P, EO], F32, tag="g")
                for ke in range(KE):
                    nc.tensor.matmul(
                        ps[:],
                        lhsT=t[:, ke, sub : sub + P],
                        rhs=wv_sb[:, ke, :],
                        start=(ke == 0),
                        stop=(ke == KE - 1),
                    )
                ps_g = ps[:].rearrange("p (h2 two d) -> p h2 two d", h2=HL // 2, two=2)
                nc.vector.tensor_copy(v_g[:, kt, :, 0, 0:D], ps_g[:, :, 0, :])
                nc.vector.tensor_copy(v_g[:, kt, :, 1, D:P], ps_g[:, :, 1, :])

            kin_bufs = {}

            def dma_kin(c, eng=None):  # stage kinT cols c*512..(c+1)*512
                t = kin_pool.tile([P, KE, 512], BF16, tag="kin")
                (eng or nc.sync).dma_start(
                    out=t[:], in_=kinT_r[:, :, c * 512 : (c + 1) * 512]
                )
                kin_bufs[c] = t

            def pk(et, c):  # K^T e-tile et for k chunk c
                t = kin_bufs[c]
                ps = g_psum.tile([P, 512], F32, tag="g")
                for ke in range(KE):
                    nc.tensor.matmul(
                        ps[:],
                        lhsT=wk_sb[:, ke, et * P : (et + 1) * P],
                        rhs=t[:, ke, :],
                        start=(ke == 0),
                        stop=(ke == KE - 1),
                    )
                if (et + c) % 2 == 0:
                    nc.vector.tensor_scalar_add(
                        kT[:, et, c * 512 : (c + 1) * 512],
                        ps[:],
                        bk_sb[:, et : et + 1],
                    )
                else:
                    nc.scalar.activation(
                        kT[:, et, c * 512 : (c + 1) * 512],
                        ps[:],
                        mybir.ActivationFunctionType.Identity,
                        bias=bk_sb[:, et : et + 1],
                    )

            qin_bufs = {}

            def dma_qin(c, eng=None):
                t = qin_pool.tile([P, KE, 512], BF16, tag="qin")
                (eng or nc.sync).dma_start(
                    out=t[:], in_=qinT_r[:, :, c * 512 : (c + 1) * 512]
                )
                qin_bufs[c] = t

            def pq(et, c):  # Q^T for heads (2et, 2et+1), q chunk c
                t = qin_bufs[c]
                ps = g_psum.tile([P, 512], F32, tag="g")
                for ke in range(KE):
                    nc.tensor.matmul(
                        ps[:],
                        lhsT=wq_sb[:, ke, et * P : (et + 1) * P],
                        rhs=t[:, ke, :],
                        start=(ke == 0),
                        stop=(ke == KE - 1),
                    )
                nc.vector.tensor_scalar_add(
                    qT[0:D, 2 * et, c * 512 : (c + 1) * 512],
                    ps[0:D, :],
                    bq_sb[0:D, et : et + 1],
                )
                nc.scalar.activation(
                    qT[D:P, 2 * et + 1, c * 512 : (c + 1) * 512],
                    ps[D:P, :],
                    mybir.ActivationFunctionType.Identity,
                    bias=bq_sb[D:P, et : et + 1],
                )

            def po(qt, ch):  # partial out rows qt*128, cols ch*512
                ps = g_psum.tile([P, 512], F32, tag="g")
                for ke in range(ETO):
                    nc.tensor.matmul(
                        ps[:],
                        lhsT=ctx_sb[:, ke, qt * P : (qt + 1) * P],
                        rhs=wo_sb[:, ke, ch * 512 : (ch + 1) * 512],
                        start=(ke == 0),
                        stop=(ke == ETO - 1),
                    )
                ot = out_pool.tile([P, 512], F32, tag="out_t")
                nc.vector.tensor_copy(ot[:], ps[:])
                nc.sync.dma_start(
                    out=out[qt * P : (qt + 1) * P, ch * 512 : (ch + 1) * 512],
                    in_=ot[:],
                )

            # ---- interleave plan ---------------------------------------
            # Production items run on the PE between PA steps; each PA step
            # is gated (by tile deps) on the productions emitted before it.
            NK2 = NKT // 2

            plan = []  # 'd*' = dma, tuple = production, ('pa',...) = step
            ENG = {"v": nc.scalar, "g": nc.gpsimd, "s": None}

            def pa_steps(qc, h, prods):
                # 8 PA steps for group (qc, h) with prods[i] emitted before
                # step i
                for j in range(NK2):
                    for it in prods[j] if j < len(prods) else []:
                        plan.append(it)
                    plan.append(("pa", h, qc, j))

            # prefix: minimal DMA critical path to the first PA step, then
            # the rest of the staging queue (sync engine issues in order)
            plan += [("dw", "wv"), ("dv", 0, "v"), ("dwp", "wk", 0, "g"),
                     ("dk", 0), ("dwp", "wq", 0, "v"), ("dq", 0, "g"),
                     ("dv", 1, "v"), ("dv", 2), ("dv", 3),
                     ("dk", 1),
                     ("dwp", "wk", 1), ("dwp", "wk", 2), ("dwp", "wk", 3),
                     ("dwp", "wq", 1), ("dwp", "wq", 2), ("dwp", "wq", 3)]
            plan += [("pv", 0), ("pv", 1), ("pk", 0, 0), ("pq", 0, 0)]
            # qc0 h0: step j consumes PV kt 2j,2j+1 and kT[0] chunk j//2;
            # PK produced c-major-ish so kin staging buffers rotate in order
            pa_steps(0, 0, [
                [("dv", 4), ("pv", 2), ("pv", 3), ("pk", 0, 1)],
                [("dv", 5), ("pv", 4), ("pv", 5), ("dk", 2), ("pk", 1, 0),
                 ("pk", 1, 1)],
                [("dv", 6), ("pv", 6), ("pv", 7), ("pk", 2, 0), ("pk", 2, 1)],
                [("dv", 7), ("pv", 8), ("pv", 9), ("pk", 3, 0), ("pk", 0, 2)],
                [("pv", 10), ("pv", 11), ("dk", 3), ("pk", 3, 1), ("pk", 1, 2)],
                [("pv", 12), ("pv", 13), ("pk", 2, 2), ("pk", 0, 3)],
                [("pv", 14), ("pv", 15), ("pk", 3, 2), ("pk", 1, 3)],
                [("pk", 2, 3)],
            ])
            pa_steps(0, 1, [[("pk", 3, 3)], [("pq", 1, 0)], [("pq", 2, 0)],
                            [("dw", "wo")], [], [], [], []])
            pa_steps(0, 2, [[("dq", 1)], [], [], [], [], [], [], []])
            pa_steps(0, 3, [[("pq", 3, 0)], [], [("pq", 0, 1)], [],
                            [("pq", 1, 1)], [], [("pq", 2, 1)], []])
            pa_steps(0, 4, [[("pq", 3, 1)], [], [("dq", 2)], [], [], [], [], []])
            pa_steps(0, 5, [[("pq", 0, 2)], [], [("pq", 1, 2)], [],
                            [("pq", 2, 2)], [], [("pq", 3, 2)], []])
            pa_steps(0, 6, [[("dq", 3)], [], [], [], [], [], [], []])
            pa_steps(0, 7, [[("pq", 0, 3)], [], [("pq", 1, 3)], [],
                            [("pq", 2, 3)], [], [("pq", 3, 3)], []])
            # qc1..3: interleave PO of the previous (completed) q chunk.
            # PO goes at step>=1 so the previous group's pending PV flush
            # (emitted inside step 0) precedes it in the PE stream.
            for qc in range(1, NQC):
                for h in range(HL):
                    qt0 = (qc - 1) * 4
                    prods = []
                    if h < 4:
                        prods = [[], [("po", qt0 + h, 0)], [], [],
                                 [("po", qt0 + h, 1)], [], [], []]
                    pa_steps(qc, h, prods)
            # tail: flush the last pending PV/normalize, then PO of qc3
            plan.append(("flush",))
            for qt in range(12, 16):
                for ch in range(2):
                    plan.append(("po", qt, ch))

            # ---- emit --------------------------------------------------
            pending = None  # (exp_t, h, qc, kt2, ctx_ps)
            ctx_map = {}

            def emit_pv_mm(exp_t, h, qc, kt2, ctx_ps):
                for j in range(2):
                    nc.tensor.matmul(
                        ctx_ps[:],
                        lhsT=v_sb[:, 2 * kt2 + j, h, :],
                        rhs=exp_t[:, j, :],
                        start=(kt2 == 0 and j == 0),
                        stop=(kt2 == NK2 - 1 and j == 1),
                    )  # out rows: ctx on head's parity, denom on the other
                if kt2 == NK2 - 1:
                    # Release ctx_ps fast: copy raw ctx + den to SBUF, then
                    # normalize in place off the PSUM critical path.
                    par = h % 2
                    dlo, dhi = (0, D) if par == 0 else (D, P)
                    den_row = D if par == 0 else 0
                    cslice = ctx_sb[dlo:dhi, h // 2, qc * 512 : (qc + 1) * 512]
                    den = nrm_pool.tile([1, 512], F32, tag="den")
                    nc.vector.tensor_copy(den[:], ctx_ps[den_row : den_row + 1, :])
                    nc.vector.tensor_copy(cslice, ctx_ps[dlo:dhi, :])
                    recip = nrm_pool.tile([1, 512], F32, tag="recip")
                    nc.vector.reciprocal_approx_fast(out=recip[:], in_=den[:])
                    rb = nrm_pool.tile([P, 512], F32, tag="rb")
                    nc.gpsimd.partition_broadcast(rb[:], recip[:])
                    nc.vector.tensor_mul(cslice, cslice, rb[dlo:dhi, :])

            for it in plan:
                kind = it[0]
                if kind == "dv":
                    dma_vin(it[1], ENG[it[2]] if len(it) > 2 else None)
                elif kind == "dk":
                    dma_kin(it[1], ENG[it[2]] if len(it) > 2 else None)
                elif kind == "dq":
                    dma_qin(it[1], ENG[it[2]] if len(it) > 2 else None)
                elif kind == "dw":
                    t, src = w_dma[it[1]]
                    nc.sync.dma_start(out=t[:], in_=src)
                elif kind == "dwp":
                    t, src = w_dma[it[1]]
                    et = it[2]
                    eng = ENG[it[3]] if len(it) > 3 else nc.sync
                    (eng or nc.sync).dma_start(
                        out=t[:, :, et * P : (et + 1) * P],
                        in_=src[:, :, et * P : (et + 1) * P],
                    )
                elif kind == "flush":
                    if pending is not None:
                        emit_pv_mm(*pending)
                        pending = None
                elif kind == "pv":
                    pv(it[1])
                elif kind == "pk":
                    pk(it[1], it[2])
                elif kind == "pq":
                    pq(it[1], it[2])
                elif kind == "po":
                    po(it[1], it[2])
                else:  # PA step
                    _, h, qc, kt2 = it
                    if kt2 == 0:
                        ctx_map[(h, qc)] = c_psum.tile(
                            [P, 512], F32, tag="ctx", name="ctx_ps"
                        )
                    ctx_ps = ctx_map[(h, qc)]
                    s_ps = s_psum.tile([P, 2, 512], F32, tag="s")
                    for j in range(2):
                        nc.tensor.matmul(
                            s_ps[:, j, :],
                            lhsT=kT[
                                :, h // 2, (2 * kt2 + j) * P : (2 * kt2 + j + 1) * P
                            ],
                            rhs=qT[:, h, qc * 512 : (qc + 1) * 512],
                            start=True,
                            stop=True,
                        )
                    exp_t = exp_pool.tile([P, 2, 512], BF16, tag="exp")
                    nc.scalar.activation(
                        exp_t[:],
                        s_ps[:],
                        mybir.ActivationFunctionType.Exp,
                        scale=0.125,
                    )
                    if pending is not None:
                        emit_pv_mm(*pending)
                    pending = (exp_t, h, qc, kt2, ctx_ps)
            if pending is not None:
                emit_pv_mm(*pending)

    nc.compile()
    return nc


def _get_program():
    if "nc" not in _CACHE:
        _CACHE["nc"] = _build_program()
    return _CACHE["nc"]


def kernel(query, key, value, Wq, Wk, Wv, Wo, bq, bk, bv, bo):
    global _LAST_RESULTS
    import ml_dtypes

    bf16 = ml_dtypes.bfloat16
    query = np.asarray(query, dtype=np.float32)
    key = np.asarray(key, dtype=np.float32)
    value = np.asarray(value, dtype=np.float32)
    Wq = np.asarray(Wq, np.float32)
    Wk = np.asarray(Wk, np.float32)
    Wv = np.asarray(Wv, np.float32)
    Wo = np.asarray(Wo, np.float32)

    in_maps = []
    for c in range(N_CORES):
        b, hh = c // 2, c % 2
        cs = slice(hh * EO, (hh + 1) * EO)
        in_maps.append(
            {
                "qinT": np.ascontiguousarray(query[b].T.astype(bf16)),
                "kinT": np.ascontiguousarray(key[b].T.astype(bf16)),
                "vinT": np.ascontiguousarray(value[b].T.astype(bf16)),
                "Wq": np.ascontiguousarray(Wq[:, cs].astype(bf16)),
                "Wk": np.ascontiguousarray(Wk[:, cs].astype(bf16)),
                "Wv": np.ascontiguousarray(Wv[:, cs].astype(bf16)),
                "Wo": np.ascontiguousarray(Wo[cs, :].astype(bf16)),
                "bq": np.ascontiguousarray(np.asarray(bq, np.float32)[cs]),
                "bk": np.ascontiguousarray(np.asarray(bk, np.float32)[cs]),
            }
        )
    nc = _get_program()
    res = run_bass_kernel_spmd(nc, in_maps, list(range(N_CORES)))
    _LAST_RESULTS = res
    row = (
        np.asarray(bv, np.float64) @ np.asarray(Wo, np.float64)
        + np.asarray(bo, np.float64)
    ).astype(np.float32)
    full = np.empty((B, S, E), dtype=np.float32)
    for b in range(B):
        full[b] = res.results[2 * b]["out"] + res.results[2 * b + 1]["out"] + row
    return full
